# revision 1
# baseline (speedup 1.0000x reference)
"""Trainium2 Bass kernel for the ConvMod problem:

    Y1 = valid 2x2 cross-correlation(X, W)    # [4095, 4095]
    Y2 = transposed-conv(Y1, W)               # [4096, 4096]

The composite equals, in the interior, a 3x3 convolution of X with
K = corr(W, W), plus boundary corrections coming from the clipping of
Y1's domain:

    Y2 = Conv3x3_zeropad(X; K) - E_row - E_col + E_both

  E_row[0, q]    = sum_{b,d} W[1,b] W[1,d] Xpad[0,    q+b-d]
  E_row[H-1, q]  = sum_{b,d} W[0,b] W[0,d] Xpad[H-1,  q+b-d]
  E_col[p, 0]    = sum_{a,c} W[a,1] W[c,1] Xpad[p+a-c, 0]
  E_col[p, L-1]  = sum_{a,c} W[a,0] W[c,0] Xpad[p+a-c, L-1]
  E_both (corners): (0,0): W[1,1]^2 X[0,0]; (0,L-1): W[1,0]^2 X[0,L-1];
                    (H-1,0): W[0,1]^2 X[H-1,0]; (H-1,L-1): W[0,0]^2 X[...].

Distribution: data-parallel over rows across 8 cores; each core gets a
[514, 4096] row slab of X with a 1-row halo on each side (zero-padded at
the global edges), plus per-core stationary band matrices, and produces
its [512, 4096] slice of Y2.  No collectives.

On-device: rows live on SBUF partitions, columns on the free axis.  For a
block of M output rows we load an X tile of Kin = M+2 rows and run, per
512-column chunk, three TensorE matmuls with tridiagonal stationary band
matrices (one per column offset v in {-1,0,+1}; the column shift is
applied on the moving operand's free-axis slice), all accumulating into
one PSUM bank.  N=1 correction matmuls fix output columns 0 and L-1.
Row-boundary corrections are baked into the per-core stationary matrices.
PSUM is evacuated to SBUF alternately on ScalarE/VectorE and DMA'd out.
Matmuls run as float32r (input data is fp32 bit-identical; the PE runs
its fast reduced-precision fp32 path; accumulation is fp32).
"""

import numpy as np

import concourse.bass as bass
from concourse import bacc
import concourse.mybir as mybir
from concourse.tile import TileContext
from concourse.bass_utils import run_bass_kernel_spmd

H = 4096
L = 4096
LEXT = 4096 + 6            # slab columns: X row + [X0, 0, X0, XL, 0, XL] staging
NCORES = 8
RPC = H // NCORES          # output rows per core: 512
SLAB = RPC + 2             # input slab rows per core (1-row halo each side)
BLOCK_MS = [126, 126, 126, 126, 8]
BLOCK_STARTS = [0, 126, 252, 378, 504]
CHUNK = 512
NCH = L // CHUNK
WPAD_K = 128
WPAD_M = 126
NMATS = 15                 # 3 sets x 5 matrices
F32 = mybir.dt.float32
F32R = mybir.dt.float32r


# ----------------------------------------------------------------------------
# Host-side stationary-matrix construction
# ----------------------------------------------------------------------------

def _make_taps(W):
    W = np.asarray(W, dtype=np.float64)
    K = np.zeros((3, 3))
    for a in range(2):
        for b in range(2):
            for c in range(2):
                for d in range(2):
                    K[a - c + 1, b - d + 1] += W[a, b] * W[c, d]
    rowtop = np.zeros(3)
    rowbot = np.zeros(3)
    for b in range(2):
        for d in range(2):
            rowtop[b - d + 1] += W[1, b] * W[1, d]
            rowbot[b - d + 1] += W[0, b] * W[0, d]
    col0 = np.zeros(3)
    colL = np.zeros(3)
    for a in range(2):
        for c in range(2):
            col0[a - c + 1] += W[a, 1] * W[c, 1]
            colL[a - c + 1] += W[a, 0] * W[c, 0]
    corners = {
        (0, 0): W[1, 1] ** 2,
        (0, 1): W[1, 0] ** 2,
        (1, 0): W[0, 1] ** 2,
        (1, 1): W[0, 0] ** 2,
    }
    return K, rowtop, rowbot, col0, colL, corners


def _build_block_mats(W, M, first_row_global, last_row_global):
    """[5, M+2, M]: bands for v=-1,0,+1 then negated C0, C_L corrections."""
    K3, rowtop, rowbot, col0, colL, corners = _make_taps(W)
    Kin = M + 2
    mats = np.zeros((5, Kin, M))
    for m in range(M):
        for u in (-1, 0, 1):
            k = m + 1 + u
            for vi, v in enumerate((-1, 0, 1)):
                mats[vi, k, m] = K3[u + 1, v + 1]
            mats[3, k, m] = -col0[u + 1]
            mats[4, k, m] = -colL[u + 1]
    if first_row_global:
        for vi, v in enumerate((-1, 0, 1)):
            mats[vi, 1, 0] -= rowtop[v + 1]
        mats[3, 1, 0] += corners[(0, 0)]
        mats[4, 1, 0] += corners[(0, 1)]
    if last_row_global:
        m = M - 1
        for vi, v in enumerate((-1, 0, 1)):
            mats[vi, m + 1, m] -= rowbot[v + 1]
        mats[3, m + 1, m] += corners[(1, 0)]
        mats[4, m + 1, m] += corners[(1, 1)]
    return mats


def _build_wstack(W, core):
    """Per-core stationary stack [128, 15*126] (k-major, clean 2D DMA)."""
    out = np.zeros((WPAD_K, 3, 5, WPAD_M), dtype=np.float32)
    b0 = _build_block_mats(W, 126, core == 0, False)
    mid = _build_block_mats(W, 126, False, False)
    b4 = _build_block_mats(W, 8, False, core == NCORES - 1)
    for w in range(5):
        out[:128, 0, w, :126] = b0[w].astype(np.float32)
        out[:128, 1, w, :126] = mid[w].astype(np.float32)
        out[:10, 2, w, :8] = b4[w].astype(np.float32)
    return out.reshape(WPAD_K, NMATS * WPAD_M)


def _make_slabs(X):
    X = np.ascontiguousarray(np.asarray(X, dtype=np.float32))
    slabs = np.zeros((NCORES, SLAB, LEXT), dtype=np.float32)
    for c in range(NCORES):
        lo = c * RPC - 1
        hi = c * RPC + RPC + 1
        src_lo = max(0, lo)
        src_hi = min(H, hi)
        slabs[c, src_lo - lo : src_hi - lo, :L] = X[src_lo:src_hi, :]
    # staging columns for the N=2 edge-fix matmuls (PSUM writes must be
    # 8B-aligned with even N, so single-column terms are expressed as
    # [col, 0] / [0, col] pairs)
    slabs[:, :, L + 0] = slabs[:, :, 0]
    slabs[:, :, L + 2] = slabs[:, :, 0]
    slabs[:, :, L + 3] = slabs[:, :, L - 1]
    slabs[:, :, L + 5] = slabs[:, :, L - 1]
    return slabs


# ----------------------------------------------------------------------------
# Device program (SPMD; identical instruction stream on all 8 cores)
# ----------------------------------------------------------------------------

def build_nc(compile=True):
    nc = bacc.Bacc()
    x_d = nc.declare_dram_parameter("xslab", [SLAB, LEXT], F32R, isOutput=False)
    w_d = nc.declare_dram_parameter("wstack", [WPAD_K, NMATS * WPAD_M], F32R, isOutput=False)
    y_d = nc.declare_dram_parameter("y", [RPC, L], F32, isOutput=True)

    with TileContext(nc) as tc:
        with (
            tc.tile_pool(name="wp", bufs=1) as wp,
            tc.tile_pool(name="xp", bufs=5) as xp,
            tc.tile_pool(name="yp", bufs=4) as yp,
            tc.tile_pool(name="pp", bufs=8, space="PSUM") as pp,
        ):
            wsb = wp.tile([WPAD_K, NMATS * WPAD_M], F32R, name="wsb")
            # set 0 first (the only stationary set the first block needs);
            # sets 1/2 are issued after block 0's load pieces below
            nc.scalar.dma_start(
                out=wsb[:, 0 : 5 * WPAD_M], in_=w_d[:, 0 : 5 * WPAD_M]
            )

            for b in (0, 1, 4, 2, 3):
                M, s = BLOCK_MS[b], BLOCK_STARTS[b]
                Kin = M + 2
                si = 0 if b == 0 else (1 if b < 4 else 2)

                xt = xp.tile([128, LEXT], F32R, name=f"xt{b}", tag="xt")
                # column-split pieces so chunk-0 matmuls start after ~1/4 of
                # the tile has landed; the tiny staging-column piece goes
                # first (the chunk-0 edge matmuls read it)
                nc.scalar.dma_start(
                    out=xt[:Kin, L:LEXT], in_=x_d[s : s + Kin, L:LEXT]
                )
                for pi, (a, b_) in enumerate(
                    ((0, 1024), (1024, 2048), (2048, 3072), (3072, L))
                ):
                    dma_eng = nc.sync if pi % 2 == 0 else nc.scalar
                    dma_eng.dma_start(
                        out=xt[:Kin, a:b_], in_=x_d[s : s + Kin, a:b_]
                    )
                if b == 0:
                    # remaining stationary sets, needed from block 2 onward
                    nc.sync.dma_start(
                        out=wsb[:, 5 * WPAD_M :], in_=w_d[:, 5 * WPAD_M :]
                    )
                yt = yp.tile([128, L], F32, name=f"yt{b}", tag="yt")

                def wm(wi):
                    base = (si * 5 + wi) * WPAD_M
                    return wsb[0:Kin, base : base + M]

                def xr(c0, n):
                    return xt[0:Kin, c0 : c0 + n]

                pts = [
                    pp.tile([128, CHUNK], F32, name=f"pt{b}_{q}", tag="pt")
                    for q in range(NCH)
                ]

                # band v=0: full-width first touch per bank (start=True
                # clears the bank; partial-width bands then accumulate)
                for q in range(NCH):
                    nc.tensor.matmul(
                        pts[q][0:M, 0:CHUNK], wm(1), xr(q * CHUNK, CHUNK),
                        start=True, stop=False,
                    )
                # band v=-1 (psum writes must be 8B-aligned, even N:
                # chunk 0 covers [2:512); cols 0-1 are fixed below)
                nc.tensor.matmul(
                    pts[0][0:M, 2:CHUNK], wm(0), xr(1, CHUNK - 2),
                    start=False, stop=False,
                )
                for q in range(1, NCH):
                    nc.tensor.matmul(
                        pts[q][0:M, 0:CHUNK], wm(0), xr(q * CHUNK - 1, CHUNK),
                        start=False, stop=False,
                    )
                # band v=+1; chunk 0 first so its evacuation starts early
                nc.tensor.matmul(
                    pts[0][0:M, 0:CHUNK], wm(2), xr(1, CHUNK),
                    start=False, stop=False,
                )
                # left-edge fixes via N=2 matmuls on the [X0, 0, X0] staging
                # cols: col 0 += C0 . X0 ; col 1 += Band_-1 . X0
                nc.tensor.matmul(
                    pts[0][0:M, 0:2], wm(3), xr(L, 2), start=False, stop=False
                )
                nc.tensor.matmul(
                    pts[0][0:M, 0:2], wm(0), xr(L + 1, 2), start=False, stop=True
                )
                for q in range(1, NCH - 1):
                    nc.tensor.matmul(
                        pts[q][0:M, 0:CHUNK], wm(2), xr(q * CHUNK + 1, CHUNK),
                        start=False, stop=True,
                    )
                nc.tensor.matmul(
                    pts[NCH - 1][0:M, 0 : CHUNK - 2],
                    wm(2), xr((NCH - 1) * CHUNK + 1, CHUNK - 2),
                    start=False, stop=False,
                )
                # right-edge fixes on [XL, 0, XL]: col 510 += Band_+1 . XL ;
                # col 511 += C_L . XL
                nc.tensor.matmul(
                    pts[NCH - 1][0:M, CHUNK - 2 : CHUNK], wm(2), xr(L + 3, 2),
                    start=False, stop=False,
                )
                nc.tensor.matmul(
                    pts[NCH - 1][0:M, CHUNK - 2 : CHUNK], wm(4), xr(L + 4, 2),
                    start=False, stop=True,
                )

                # evacuate PSUM -> SBUF on two engines, then DMA out
                for q in range(NCH):
                    src = pts[q][0:M, 0:CHUNK]
                    dst = yt[0:M, q * CHUNK : (q + 1) * CHUNK]
                    if q in (0, 4, 6):
                        nc.scalar.copy(dst, src)
                    else:
                        nc.vector.tensor_copy(dst, src)
                # stores on SWDGE (GpSimd issue path is idle), in pieces
                # so they drain while later chunks still compute
                for a in range(0, L, 1024):
                    nc.gpsimd.dma_start(
                        out=y_d[s : s + M, a : a + 1024],
                        in_=yt[0:M, a : a + 1024],
                    )
    if compile:
        nc.compile()
    return nc


_NC_CACHE = None


def _get_nc():
    global _NC_CACHE
    if _NC_CACHE is None:
        _NC_CACHE = build_nc()
    return _NC_CACHE


def _run(X, W, trace=False, **spmd_kwargs):
    slabs = _make_slabs(X)
    in_maps = []
    for c in range(NCORES):
        in_maps.append(
            {"xslab": slabs[c], "wstack": _build_wstack(W, c)}
        )
    res = run_bass_kernel_spmd(
        _get_nc(), in_maps, core_ids=list(range(NCORES)), trace=trace, **spmd_kwargs
    )
    Y = np.concatenate([res.results[c]["y"] for c in range(NCORES)], axis=0)
    return Y, res


def kernel(X, W):
    Y, _ = _run(X, W)
    return Y



# revision 4
# speedup vs baseline: 1.4796x; 1.4796x over previous
"""Trainium2 Bass kernel for the ConvMod problem:

    Y1 = valid 2x2 cross-correlation(X, W)    # [4095, 4095]
    Y2 = transposed-conv(Y1, W)               # [4096, 4096]

The composite equals Y2 = Conv3x3_zeropad(X; K) - E_row - E_col + E_corner
with K = corr(W, W).  The E corrections only touch the first/last global
row and column, so they are applied on the HOST (O(H) numpy work); the
device computes the pure zero-padded 3x3 convolution, which is perfectly
uniform -- no per-block or per-core special cases.

Distribution: column-parallel across 8 cores.  Each core owns a
[4096, 512] column stripe of the output and reads a [4098, 514] fp16
input slab (1-col halo each side, 1 zero row top/bottom; halos at global
edges are zero).  fp16 I/O halves HBM traffic vs fp32 (tolerance is
2e-2; fp16 gives ~1e-3).

On-device: rows on SBUF partitions, columns on the free axis.  33 row
tiles per core (32 x 126 rows + 1 x 64), each computed by 3 TensorE
band matmuls (one per column shift v in {-1,0,+1}) accumulating into one
PSUM bank; all matmuls are full width N=512.  Matmuls are grouped 8
tiles at a time, v-major, so consecutive matmuls share the same
stationary band matrix.  PSUM is evacuated to fp16 SBUF alternately on
ScalarE/VectorE and stored with 5 large DMAs in a block-major HBM
layout that the host un-permutes.
"""

import numpy as np

import concourse.bass as bass
from concourse import bacc
import concourse.mybir as mybir
from concourse.tile import TileContext
from concourse.bass_utils import run_bass_kernel_spmd

H = 4096
L = 4096
NCORES = 8
CPC = L // NCORES          # output columns per core: 512
SLABW = CPC + 2            # input slab cols (1-col halo each side)
SLABH = H + 2              # input slab rows (1 zero row top+bottom)
M_MAIN = 126               # output rows per tile (Kin = 128)
NT = 33                    # 32 full tiles + 1 tail tile
M_TAIL = H - 32 * M_MAIN   # 64
GROUP = 8                  # tiles per PSUM/stationary-reuse group
F32 = mybir.dt.float32
F16 = mybir.dt.float16
WCOLS = 3 * M_MAIN + 3 * M_TAIL  # stationary stack free width: 570


def _tile_m(t):
    return M_MAIN if t < NT - 1 else M_TAIL


# ----------------------------------------------------------------------------
# Host-side tap / stationary-matrix construction
# ----------------------------------------------------------------------------

def _make_taps(W):
    """K = corr2d(W, W) (3x3) plus the 3-tap boundary correction filters."""
    W = np.asarray(W, dtype=np.float64)
    K = np.zeros((3, 3))
    for a in range(2):
        for b in range(2):
            for c in range(2):
                for d in range(2):
                    K[a - c + 1, b - d + 1] += W[a, b] * W[c, d]
    rowtop = np.zeros(3)
    rowbot = np.zeros(3)
    for b in range(2):
        for d in range(2):
            rowtop[b - d + 1] += W[1, b] * W[1, d]
            rowbot[b - d + 1] += W[0, b] * W[0, d]
    col0 = np.zeros(3)
    colL = np.zeros(3)
    for a in range(2):
        for c in range(2):
            col0[a - c + 1] += W[a, 1] * W[c, 1]
            colL[a - c + 1] += W[a, 0] * W[c, 0]
    corners = {
        (0, 0): W[1, 1] ** 2,
        (0, 1): W[1, 0] ** 2,
        (1, 0): W[0, 1] ** 2,
        (1, 1): W[0, 0] ** 2,
    }
    return K, rowtop, rowbot, col0, colL, corners


def _build_wstack(W):
    """[128, 570] fp16: three [128,126] band matrices (v=0,1,2) for the
    main tiles followed by three [66,64] bands for the tail tile.
    B_v[m+u, m] = K[u, v]."""
    K, *_ = _make_taps(W)
    out = np.zeros((128, WCOLS), dtype=np.float32)
    for v in range(3):
        for u in range(3):
            for m in range(M_MAIN):
                out[m + u, v * M_MAIN + m] = K[u, v]
            for m in range(M_TAIL):
                out[m + u, 3 * M_MAIN + v * M_TAIL + m] = K[u, v]
    return out.astype(np.float16)


def _make_slabs(X):
    """[8, 4098, 514] fp16 column stripes with halos / zero padding."""
    Xh = np.asarray(X, dtype=np.float32).astype(np.float16)
    slabs = np.zeros((NCORES, SLABH, SLABW), dtype=np.float16)
    for c in range(NCORES):
        lo = c * CPC - 1
        hi = c * CPC + CPC + 1
        src_lo = max(0, lo)
        src_hi = min(L, hi)
        slabs[c, 1 : H + 1, src_lo - lo : src_hi - lo] = Xh[:, src_lo:src_hi]
    return slabs


def _host_edge_fix(Y, X):
    """Subtract the clipping corrections on the global boundary rows/cols
    (in place, float64 filters on float32 X)."""
    _, rowtop, rowbot, col0, colL, corners = _make_taps(
        _host_edge_fix.W  # set by caller
    )

    def filt(x, t):
        xz = np.zeros(x.shape[0] + 2, dtype=np.float64)
        xz[1:-1] = x
        return t[0] * xz[:-2] + t[1] * xz[1:-1] + t[2] * xz[2:]

    X = np.asarray(X, dtype=np.float64)
    Y[0, :] -= filt(X[0, :], rowtop)
    Y[-1, :] -= filt(X[-1, :], rowbot)
    Y[:, 0] -= filt(X[:, 0], col0)
    Y[:, -1] -= filt(X[:, -1], colL)
    Y[0, 0] += corners[(0, 0)] * X[0, 0]
    Y[0, -1] += corners[(0, 1)] * X[0, -1]
    Y[-1, 0] += corners[(1, 0)] * X[-1, 0]
    Y[-1, -1] += corners[(1, 1)] * X[-1, -1]
    return Y


# ----------------------------------------------------------------------------
# Device program (SPMD; identical instruction stream and stationary data
# on all 8 cores)
# ----------------------------------------------------------------------------

def build_nc(compile=True):
    nc = bacc.Bacc()
    x_d = nc.declare_dram_parameter("xslab", [SLABH, SLABW], F16, isOutput=False)
    w_d = nc.declare_dram_parameter("wstack", [128, WCOLS], F16, isOutput=False)
    # block-major output: block t lives at columns [512t, 512t+512)
    y_d = nc.declare_dram_parameter("y", [M_MAIN, NT * CPC], F16, isOutput=True)

    with TileContext(nc) as tc:
        with (
            tc.tile_pool(name="wp", bufs=1) as wp,
            tc.tile_pool(name="xp", bufs=1) as xp,
            tc.tile_pool(name="yp", bufs=1) as yp,
            tc.tile_pool(name="pp", bufs=GROUP, space="PSUM") as pp,
        ):
            wsb = wp.tile([128, WCOLS], F16, name="wsb")
            nc.scalar.dma_start(out=wsb[:, :], in_=w_d[:, :])

            xall = xp.tile([128, NT * SLABW], F16, name="xall")
            yall = yp.tile([M_MAIN, NT * CPC], F16, name="yall")

            # whole input stream as per-tile DMAs on the Sync queue
            for t in range(NT):
                kin = _tile_m(t) + 2
                nc.sync.dma_start(
                    out=xall[0:kin, t * SLABW : t * SLABW + SLABW],
                    in_=x_d[t * M_MAIN : t * M_MAIN + kin, :],
                )

            def wm(t, v):
                if t < NT - 1:
                    return wsb[0:128, v * M_MAIN : v * M_MAIN + M_MAIN]
                base = 3 * M_MAIN + v * M_TAIL
                return wsb[0 : M_TAIL + 2, base : base + M_TAIL]

            def xr(t, v):
                kin = _tile_m(t) + 2
                return xall[0:kin, t * SLABW + v : t * SLABW + v + CPC]

            pts = [None] * NT
            ngroups = (NT + GROUP - 1) // GROUP
            for g in range(ngroups):
                ts = range(g * GROUP, min((g + 1) * GROUP, NT))
                for t in ts:
                    pts[t] = pp.tile([128, CPC], F32, name=f"pt{t}", tag="pt")
                # v-major: 8 consecutive matmuls share one stationary
                for v in range(3):
                    for t in ts:
                        nc.tensor.matmul(
                            pts[t][0 : _tile_m(t), 0:CPC],
                            wm(t, v),
                            xr(t, v),
                            start=(v == 0),
                            stop=(v == 2),
                        )
                # evacuate PSUM -> fp16 SBUF on two engines
                for t in ts:
                    src = pts[t][0:M_MAIN, 0:CPC]
                    dst = yall[0:M_MAIN, t * CPC : (t + 1) * CPC]
                    if t % 2 == 0:
                        nc.scalar.copy(dst, src)
                    else:
                        nc.vector.tensor_copy(dst, src)
                # one large store per group on SWDGE
                c0 = g * GROUP * CPC
                c1 = min((g + 1) * GROUP, NT) * CPC
                nc.gpsimd.dma_start(
                    out=y_d[0:M_MAIN, c0:c1], in_=yall[0:M_MAIN, c0:c1]
                )
    if compile:
        nc.compile()
    return nc


_NC_CACHE = None


def _get_nc():
    global _NC_CACHE
    if _NC_CACHE is None:
        _NC_CACHE = build_nc()
    return _NC_CACHE


def _run(X, W, trace=False, **spmd_kwargs):
    X = np.asarray(X)
    W = np.asarray(W)
    slabs = _make_slabs(X)
    wstack = _build_wstack(W)
    in_maps = [{"xslab": slabs[c], "wstack": wstack} for c in range(NCORES)]
    res = run_bass_kernel_spmd(
        _get_nc(), in_maps, core_ids=list(range(NCORES)), trace=trace, **spmd_kwargs
    )
    Y = np.empty((H, L), dtype=np.float32)
    for c in range(NCORES):
        yc = np.asarray(res.results[c]["y"])  # [126, 33*512] fp16
        blk = yc.reshape(M_MAIN, NT, CPC).astype(np.float32)
        for t in range(NT):
            m = _tile_m(t)
            Y[t * M_MAIN : t * M_MAIN + m, c * CPC : (c + 1) * CPC] = blk[:m, t]
    _host_edge_fix.W = W
    _host_edge_fix(Y, X)
    return Y, res


def kernel(X, W):
    Y, _ = _run(X, W)
    return Y


# revision 8
# speedup vs baseline: 1.7342x; 1.1721x over previous
"""Trainium2 Bass kernel for the ConvMod problem:

    Y1 = valid 2x2 cross-correlation(X, W)    # [4095, 4095]
    Y2 = transposed-conv(Y1, W)               # [4096, 4096]

The composite equals Y2 = Conv3x3_zeropad(X; K) - E_row - E_col + E_corner
with K = corr(W, W).  The E corrections only touch the first/last global
row and column, so they are applied on the HOST (O(H) numpy work); the
device computes the pure zero-padded 3x3 convolution, which is perfectly
uniform -- no per-block or per-core special cases.

Distribution: column-parallel across 8 cores.  Each core owns a
[4096, 512] column stripe of the output and reads a [4098, 514] fp16
input slab (1-col halo each side, 1 zero row top/bottom; halos at global
edges are zero).  fp16 I/O halves HBM traffic vs fp32 (tolerance is
2e-2; fp16 gives ~1e-3).

On-device: rows on SBUF partitions, columns on the free axis.  33 row
tiles per core (32 x 126 rows + 1 x 64), each computed by 3 TensorE
band matmuls (one per column shift v in {-1,0,+1}) accumulating into one
PSUM bank; all matmuls are full width N=512.  Matmuls are grouped 8
tiles at a time, v-major, so consecutive matmuls share the same
stationary band matrix.  PSUM is evacuated to fp16 SBUF alternately on
ScalarE/VectorE and stored with 5 large DMAs in a block-major HBM
layout that the host un-permutes.
"""

import numpy as np

import concourse.bass as bass
from concourse import bacc
import concourse.mybir as mybir
from concourse.tile import TileContext
from concourse.bass_utils import run_bass_kernel_spmd

H = 4096
L = 4096
NCORES = 8
CPC = L // NCORES          # output columns per core: 512
SLABW = CPC + 2            # input slab cols (1-col halo each side)
SLABH = H + 2              # input slab rows (1 zero row top+bottom)
M_MAIN = 126               # output rows per tile (Kin = 128)
NT = 33                    # 32 full tiles + 1 tail tile
M_TAIL = H - 32 * M_MAIN   # 64
GROUP = 8                  # tiles per PSUM/stationary-reuse group
F32 = mybir.dt.float32
F16 = mybir.dt.float16
WCOLS = 3 * M_MAIN + 3 * M_TAIL  # stationary stack free width: 570


def _tile_m(t):
    return M_MAIN if t < NT - 1 else M_TAIL


# ----------------------------------------------------------------------------
# Host-side tap / stationary-matrix construction
# ----------------------------------------------------------------------------

def _make_taps(W):
    """K = corr2d(W, W) (3x3) plus the 3-tap boundary correction filters."""
    W = np.asarray(W, dtype=np.float64)
    K = np.zeros((3, 3))
    for a in range(2):
        for b in range(2):
            for c in range(2):
                for d in range(2):
                    K[a - c + 1, b - d + 1] += W[a, b] * W[c, d]
    rowtop = np.zeros(3)
    rowbot = np.zeros(3)
    for b in range(2):
        for d in range(2):
            rowtop[b - d + 1] += W[1, b] * W[1, d]
            rowbot[b - d + 1] += W[0, b] * W[0, d]
    col0 = np.zeros(3)
    colL = np.zeros(3)
    for a in range(2):
        for c in range(2):
            col0[a - c + 1] += W[a, 1] * W[c, 1]
            colL[a - c + 1] += W[a, 0] * W[c, 0]
    corners = {
        (0, 0): W[1, 1] ** 2,
        (0, 1): W[1, 0] ** 2,
        (1, 0): W[0, 1] ** 2,
        (1, 1): W[0, 0] ** 2,
    }
    return K, rowtop, rowbot, col0, colL, corners


def _build_wstack(W):
    """[128, 570] fp16: three [128,126] band matrices (v=0,1,2) for the
    main tiles followed by three [66,64] bands for the tail tile.
    B_v[m+u, m] = K[u, v]."""
    K, *_ = _make_taps(W)
    out = np.zeros((128, WCOLS), dtype=np.float32)
    for v in range(3):
        for u in range(3):
            for m in range(M_MAIN):
                out[m + u, v * M_MAIN + m] = K[u, v]
            for m in range(M_TAIL):
                out[m + u, 3 * M_MAIN + v * M_TAIL + m] = K[u, v]
    return out.astype(np.float16)


def _make_slabs(X):
    """[8, 4098, 514] fp16 column stripes with halos / zero padding."""
    Xh = np.asarray(X, dtype=np.float32).astype(np.float16)
    slabs = np.zeros((NCORES, SLABH, SLABW), dtype=np.float16)
    for c in range(NCORES):
        lo = c * CPC - 1
        hi = c * CPC + CPC + 1
        src_lo = max(0, lo)
        src_hi = min(L, hi)
        slabs[c, 1 : H + 1, src_lo - lo : src_hi - lo] = Xh[:, src_lo:src_hi]
    return slabs


def _host_edge_fix(Y, X):
    """Subtract the clipping corrections on the global boundary rows/cols
    (in place, float64 filters on float32 X)."""
    _, rowtop, rowbot, col0, colL, corners = _make_taps(
        _host_edge_fix.W  # set by caller
    )

    def filt(x, t):
        xz = np.zeros(x.shape[0] + 2, dtype=np.float64)
        xz[1:-1] = x
        return t[0] * xz[:-2] + t[1] * xz[1:-1] + t[2] * xz[2:]

    X = np.asarray(X, dtype=np.float64)
    Y[0, :] -= filt(X[0, :], rowtop)
    Y[-1, :] -= filt(X[-1, :], rowbot)
    Y[:, 0] -= filt(X[:, 0], col0)
    Y[:, -1] -= filt(X[:, -1], colL)
    Y[0, 0] += corners[(0, 0)] * X[0, 0]
    Y[0, -1] += corners[(0, 1)] * X[0, -1]
    Y[-1, 0] += corners[(1, 0)] * X[-1, 0]
    Y[-1, -1] += corners[(1, 1)] * X[-1, -1]
    return Y


# ----------------------------------------------------------------------------
# Device program (SPMD; identical instruction stream and stationary data
# on all 8 cores)
# ----------------------------------------------------------------------------

def build_nc(compile=True):
    nc = bacc.Bacc()
    x_d = nc.declare_dram_parameter("xslab", [SLABH, SLABW], F16, isOutput=False)
    w_d = nc.declare_dram_parameter("wstack", [128, WCOLS], F16, isOutput=False)
    # block-major output: block t lives at columns [512t, 512t+512)
    y_d = nc.declare_dram_parameter("y", [M_MAIN, NT * CPC], F16, isOutput=True)

    with TileContext(nc) as tc:
        with (
            tc.tile_pool(name="wp", bufs=1) as wp,
            tc.tile_pool(name="xp", bufs=1) as xp,
            tc.tile_pool(name="yp", bufs=1) as yp,
            tc.tile_pool(name="pp", bufs=GROUP, space="PSUM") as pp,
        ):
            wsb = wp.tile([128, WCOLS], F16, name="wsb")
            nc.scalar.dma_start(out=wsb[:, :], in_=w_d[:, :])

            xall = xp.tile([128, NT * SLABW], F16, name="xall")
            yall = yp.tile([M_MAIN, NT * CPC], F16, name="yall")

            # Batched input DMAs: B full tiles per dma_start via hand-built
            # 3D access patterns (HBM iterates (tile, row, col); SBUF
            # matches with the partition dim in the middle).  Batches
            # alternate between the Sync and Vector HWDGE queues so issue
            # cost (~0.7us per DMA on one SEQ) never throttles the stream.
            APc = bass.AP
            xrow = NT * SLABW  # sbuf partition stride (flat row width)

            def load_batch(eng, t0, nb):
                # iteration order (partition/row, tile, col) on both sides
                hbm = APc(
                    x_d[0:1, 0:1].tensor,
                    t0 * M_MAIN * SLABW,
                    [[SLABW, 128], [M_MAIN * SLABW, nb], [1, SLABW]],
                )
                sb = APc(
                    xall[0:1, 0:1].tensor,
                    t0 * SLABW,
                    [[xrow, 128], [SLABW, nb], [1, SLABW]],
                )
                eng.dma_start(out=sb, in_=hbm)

            batches = [2, 2, 4, 4, 4, 4, 4, 4, 4]
            t0 = 0
            for nb in batches:
                load_batch(nc.sync, t0, nb)
                t0 += nb
            # tail tile (Kin = 66)
            nc.sync.dma_start(
                out=xall[0 : M_TAIL + 2, (NT - 1) * SLABW : NT * SLABW],
                in_=x_d[(NT - 1) * M_MAIN : (NT - 1) * M_MAIN + M_TAIL + 2, :],
            )

            def wm(t, v):
                if t < NT - 1:
                    return wsb[0:128, v * M_MAIN : v * M_MAIN + M_MAIN]
                base = 3 * M_MAIN + v * M_TAIL
                return wsb[0 : M_TAIL + 2, base : base + M_TAIL]

            def xr(t, v):
                kin = _tile_m(t) + 2
                return xall[0:kin, t * SLABW + v : t * SLABW + v + CPC]

            pts = [None] * NT
            ngroups = (NT + GROUP - 1) // GROUP
            for g in range(ngroups):
                ts = range(g * GROUP, min((g + 1) * GROUP, NT))
                for t in ts:
                    pts[t] = pp.tile([128, CPC], F32, name=f"pt{t}", tag="pt")
                # v-major: 8 consecutive matmuls share one stationary
                for v in range(3):
                    for t in ts:
                        nc.tensor.matmul(
                            pts[t][0 : _tile_m(t), 0:CPC],
                            wm(t, v),
                            xr(t, v),
                            start=(v == 0),
                            stop=(v == 2),
                        )
                # evacuate PSUM -> fp16 SBUF on two engines
                for t in ts:
                    src = pts[t][0:M_MAIN, 0:CPC]
                    dst = yall[0:M_MAIN, t * CPC : (t + 1) * CPC]
                    if t % 2 == 0:
                        nc.scalar.copy(dst, src)
                    else:
                        nc.vector.tensor_copy(dst, src)

            # Output stores on the Sync HWDGE queue (it is done issuing
            # input batches by the time the first group is evacuated).
            # Trailing groups shrink so the final store is small.
            out_groups = [4, 4, 4, 4, 4, 4, 4, 2, 2]
            t0 = 0
            for nb in out_groups:
                c0, c1 = t0 * CPC, (t0 + nb) * CPC
                nc.sync.dma_start(
                    out=y_d[0:M_MAIN, c0:c1], in_=yall[0:M_MAIN, c0:c1]
                )
                t0 += nb
            # tail tile: only its 64 valid rows
            c0 = (NT - 1) * CPC
            nc.sync.dma_start(
                out=y_d[0:M_TAIL, c0 : c0 + CPC],
                in_=yall[0:M_TAIL, c0 : c0 + CPC],
            )
    if compile:
        nc.compile()
    return nc


_NC_CACHE = None


def _get_nc():
    global _NC_CACHE
    if _NC_CACHE is None:
        _NC_CACHE = build_nc()
    return _NC_CACHE


def _run(X, W, trace=False, **spmd_kwargs):
    X = np.asarray(X)
    W = np.asarray(W)
    slabs = _make_slabs(X)
    wstack = _build_wstack(W)
    in_maps = [{"xslab": slabs[c], "wstack": wstack} for c in range(NCORES)]
    res = run_bass_kernel_spmd(
        _get_nc(), in_maps, core_ids=list(range(NCORES)), trace=trace, **spmd_kwargs
    )
    Y = np.empty((H, L), dtype=np.float32)
    for c in range(NCORES):
        yc = np.asarray(res.results[c]["y"])  # [126, 33*512] fp16
        blk = yc.reshape(M_MAIN, NT, CPC).astype(np.float32)
        for t in range(NT):
            m = _tile_m(t)
            Y[t * M_MAIN : t * M_MAIN + m, c * CPC : (c + 1) * CPC] = blk[:m, t]
    _host_edge_fix.W = W
    _host_edge_fix(Y, X)
    return Y, res


def kernel(X, W):
    Y, _ = _run(X, W)
    return Y


# revision 11
# speedup vs baseline: 1.8157x; 1.0470x over previous
"""Trainium2 Bass kernel for the ConvMod problem:

    Y1 = valid 2x2 cross-correlation(X, W)    # [4095, 4095]
    Y2 = transposed-conv(Y1, W)               # [4096, 4096]

The composite equals Y2 = Conv3x3_zeropad(X; K) - E_row - E_col + E_corner
with K = corr(W, W).  The E corrections only touch the first/last global
row and column, so they are applied on the HOST (O(H) numpy work); the
device computes the pure zero-padded 3x3 convolution, which is perfectly
uniform -- no per-block or per-core special cases.

Distribution: column-parallel across 8 cores.  Each core owns a
[4096, 512] column stripe of the output and reads a [4098, 514] fp16
input slab (1-col halo each side, 1 zero row top/bottom; halos at global
edges are zero).  fp16 I/O halves HBM traffic vs fp32 (tolerance is
2e-2; fp16 gives ~1e-3).

On-device: rows on SBUF partitions, columns on the free axis.  33 row
tiles per core (32 x 126 rows + 1 x 64), each computed by 3 TensorE
band matmuls (one per column shift v in {-1,0,+1}) accumulating into one
PSUM bank; all matmuls are full width N=512.  Matmuls are grouped 8
tiles at a time, v-major, so consecutive matmuls share the same
stationary band matrix.  PSUM is evacuated to fp16 SBUF alternately on
ScalarE/VectorE and stored with 5 large DMAs in a block-major HBM
layout that the host un-permutes.
"""

import numpy as np

import concourse.bass as bass
from concourse import bacc
import concourse.mybir as mybir
from concourse.tile import TileContext
from concourse.bass_utils import run_bass_kernel_spmd

H = 4096
L = 4096
NCORES = 8
CPC = L // NCORES          # output columns per core: 512
SLABW = CPC + 2            # input slab cols (1-col halo each side)
SLABH = H + 2              # input slab rows (1 zero row top+bottom)
M_MAIN = 126               # output rows per tile (Kin = 128)
NT = 33                    # 32 full tiles + 1 tail tile
M_TAIL = H - 32 * M_MAIN   # 64
GROUP = 8                  # tiles per PSUM/stationary-reuse group
F32 = mybir.dt.float32
F16 = mybir.dt.float16
WCOLS = 3 * M_MAIN + 3 * M_TAIL  # stationary stack free width: 570


def _tile_m(t):
    return M_MAIN if t < NT - 1 else M_TAIL


# ----------------------------------------------------------------------------
# Host-side tap / stationary-matrix construction
# ----------------------------------------------------------------------------

def _make_taps(W):
    """K = corr2d(W, W) (3x3) plus the 3-tap boundary correction filters."""
    W = np.asarray(W, dtype=np.float64)
    K = np.zeros((3, 3))
    for a in range(2):
        for b in range(2):
            for c in range(2):
                for d in range(2):
                    K[a - c + 1, b - d + 1] += W[a, b] * W[c, d]
    rowtop = np.zeros(3)
    rowbot = np.zeros(3)
    for b in range(2):
        for d in range(2):
            rowtop[b - d + 1] += W[1, b] * W[1, d]
            rowbot[b - d + 1] += W[0, b] * W[0, d]
    col0 = np.zeros(3)
    colL = np.zeros(3)
    for a in range(2):
        for c in range(2):
            col0[a - c + 1] += W[a, 1] * W[c, 1]
            colL[a - c + 1] += W[a, 0] * W[c, 0]
    corners = {
        (0, 0): W[1, 1] ** 2,
        (0, 1): W[1, 0] ** 2,
        (1, 0): W[0, 1] ** 2,
        (1, 1): W[0, 0] ** 2,
    }
    return K, rowtop, rowbot, col0, colL, corners


def _build_wstack(W):
    """[128, 570] fp16: three [128,126] band matrices (v=0,1,2) for the
    main tiles followed by three [66,64] bands for the tail tile.
    B_v[m+u, m] = K[u, v]."""
    K, *_ = _make_taps(W)
    out = np.zeros((128, WCOLS), dtype=np.float32)
    for v in range(3):
        for u in range(3):
            for m in range(M_MAIN):
                out[m + u, v * M_MAIN + m] = K[u, v]
            for m in range(M_TAIL):
                out[m + u, 3 * M_MAIN + v * M_TAIL + m] = K[u, v]
    return out.astype(np.float16)


def _make_slabs(X):
    """[8, 4098, 514] fp16 column stripes with halos / zero padding."""
    Xh = np.asarray(X, dtype=np.float32).astype(np.float16)
    slabs = np.zeros((NCORES, SLABH, SLABW), dtype=np.float16)
    for c in range(NCORES):
        lo = c * CPC - 1
        hi = c * CPC + CPC + 1
        src_lo = max(0, lo)
        src_hi = min(L, hi)
        slabs[c, 1 : H + 1, src_lo - lo : src_hi - lo] = Xh[:, src_lo:src_hi]
    return slabs


def _host_edge_fix(Y, X):
    """Subtract the clipping corrections on the global boundary rows/cols
    (in place, float64 filters on float32 X)."""
    _, rowtop, rowbot, col0, colL, corners = _make_taps(
        _host_edge_fix.W  # set by caller
    )

    def filt(x, t):
        xz = np.zeros(x.shape[0] + 2, dtype=np.float64)
        xz[1:-1] = x
        return t[0] * xz[:-2] + t[1] * xz[1:-1] + t[2] * xz[2:]

    X = np.asarray(X, dtype=np.float64)
    Y[0, :] -= filt(X[0, :], rowtop)
    Y[-1, :] -= filt(X[-1, :], rowbot)
    Y[:, 0] -= filt(X[:, 0], col0)
    Y[:, -1] -= filt(X[:, -1], colL)
    Y[0, 0] += corners[(0, 0)] * X[0, 0]
    Y[0, -1] += corners[(0, 1)] * X[0, -1]
    Y[-1, 0] += corners[(1, 0)] * X[-1, 0]
    Y[-1, -1] += corners[(1, 1)] * X[-1, -1]
    return Y


# ----------------------------------------------------------------------------
# Device program (SPMD; identical instruction stream and stationary data
# on all 8 cores)
# ----------------------------------------------------------------------------

def build_nc(compile=True):
    nc = bacc.Bacc()
    x_d = nc.declare_dram_parameter("xslab", [SLABH, SLABW], F16, isOutput=False)
    w_d = nc.declare_dram_parameter("wstack", [128, WCOLS], F16, isOutput=False)
    # block-major output: block t lives at columns [512t, 512t+512)
    y_d = nc.declare_dram_parameter("y", [M_MAIN, NT * CPC], F16, isOutput=True)

    with TileContext(nc) as tc:
        with (
            tc.tile_pool(name="wp", bufs=1) as wp,
            tc.tile_pool(name="xp", bufs=1) as xp,
            tc.tile_pool(name="yp", bufs=1) as yp,
            tc.tile_pool(name="pp", bufs=GROUP, space="PSUM") as pp,
        ):
            wsb = wp.tile([128, WCOLS], F16, name="wsb")
            # split so the first matmul's stationary (B_0) lands first
            nc.scalar.dma_start(
                out=wsb[:, 0:M_MAIN], in_=w_d[:, 0:M_MAIN]
            )
            nc.scalar.dma_start(
                out=wsb[:, M_MAIN:WCOLS], in_=w_d[:, M_MAIN:WCOLS]
            )

            xall = xp.tile([128, NT * SLABW], F16, name="xall")
            yall = yp.tile([M_MAIN, NT * CPC], F16, name="yall")

            # Batched input DMAs: B full tiles per dma_start via hand-built
            # 3D access patterns (HBM iterates (tile, row, col); SBUF
            # matches with the partition dim in the middle).  Batches
            # alternate between the Sync and Vector HWDGE queues so issue
            # cost (~0.7us per DMA on one SEQ) never throttles the stream.
            APc = bass.AP
            xrow = NT * SLABW  # sbuf partition stride (flat row width)

            def load_batch(eng, t0, nb):
                # iteration order (partition/row, tile, col) on both sides
                hbm = APc(
                    x_d[0:1, 0:1].tensor,
                    t0 * M_MAIN * SLABW,
                    [[SLABW, 128], [M_MAIN * SLABW, nb], [1, SLABW]],
                )
                sb = APc(
                    xall[0:1, 0:1].tensor,
                    t0 * SLABW,
                    [[xrow, 128], [SLABW, nb], [1, SLABW]],
                )
                eng.dma_start(out=sb, in_=hbm)

            batches = [1, 1, 2, 4, 4, 4, 4, 4, 4, 4]
            t0 = 0
            for nb in batches:
                load_batch(nc.sync, t0, nb)
                t0 += nb
            # tail tile (Kin = 66)
            nc.sync.dma_start(
                out=xall[0 : M_TAIL + 2, (NT - 1) * SLABW : NT * SLABW],
                in_=x_d[(NT - 1) * M_MAIN : (NT - 1) * M_MAIN + M_TAIL + 2, :],
            )

            def wm(t, v):
                if t < NT - 1:
                    return wsb[0:128, v * M_MAIN : v * M_MAIN + M_MAIN]
                base = 3 * M_MAIN + v * M_TAIL
                return wsb[0 : M_TAIL + 2, base : base + M_TAIL]

            def xr(t, v):
                kin = _tile_m(t) + 2
                return xall[0:kin, t * SLABW + v : t * SLABW + v + CPC]

            # t-major: each tile runs its 3 band matmuls back-to-back, is
            # evacuated immediately (Scalar/Vector alternating), and every
            # pair of tiles is stored right away on the Sync HWDGE queue so
            # the store stream finishes with the compute instead of after it.
            for t in range(NT):
                pt = pp.tile([128, CPC], F32, name=f"pt{t}", tag="pt")
                m = _tile_m(t)
                for v in range(3):
                    nc.tensor.matmul(
                        pt[0:m, 0:CPC],
                        wm(t, v),
                        xr(t, v),
                        start=(v == 0),
                        stop=(v == 2),
                    )
                src = pt[0:M_MAIN, 0:CPC]
                dst = yall[0:M_MAIN, t * CPC : (t + 1) * CPC]
                if t % 2 == 0:
                    nc.scalar.copy(dst, src)
                else:
                    nc.vector.tensor_copy(dst, src)
                if t % 2 == 1:
                    c0, c1 = (t - 1) * CPC, (t + 1) * CPC
                    nc.sync.dma_start(
                        out=y_d[0:M_MAIN, c0:c1], in_=yall[0:M_MAIN, c0:c1]
                    )
            # tail tile store: only its 64 valid rows
            c0 = (NT - 1) * CPC
            nc.sync.dma_start(
                out=y_d[0:M_TAIL, c0 : c0 + CPC],
                in_=yall[0:M_TAIL, c0 : c0 + CPC],
            )
    if compile:
        nc.compile()
    return nc


_NC_CACHE = None


def _get_nc():
    global _NC_CACHE
    if _NC_CACHE is None:
        _NC_CACHE = build_nc()
    return _NC_CACHE


def _run(X, W, trace=False, **spmd_kwargs):
    X = np.asarray(X)
    W = np.asarray(W)
    slabs = _make_slabs(X)
    wstack = _build_wstack(W)
    in_maps = [{"xslab": slabs[c], "wstack": wstack} for c in range(NCORES)]
    res = run_bass_kernel_spmd(
        _get_nc(), in_maps, core_ids=list(range(NCORES)), trace=trace, **spmd_kwargs
    )
    Y = np.empty((H, L), dtype=np.float32)
    for c in range(NCORES):
        yc = np.asarray(res.results[c]["y"])  # [126, 33*512] fp16
        blk = yc.reshape(M_MAIN, NT, CPC).astype(np.float32)
        for t in range(NT):
            m = _tile_m(t)
            Y[t * M_MAIN : t * M_MAIN + m, c * CPC : (c + 1) * CPC] = blk[:m, t]
    _host_edge_fix.W = W
    _host_edge_fix(Y, X)
    return Y, res


def kernel(X, W):
    Y, _ = _run(X, W)
    return Y
